# revision 1
# baseline (speedup 1.0000x reference)
"""Trainium2 Bass kernel for the CherryAllocation NAGNN (grid GIN + MLP head).

Self-contained: hardcodes shapes/sharding. Data-parallel over batch:
64 samples -> 8 NeuronCores x 8 samples. Weights replicated.

Math per sample (grid 32x32, N=1024 nodes):
  mask = obs[:1024] != 0 ; x = obs[1024:].reshape(1024, 32)
  h0 = x
  for l in 0..3:  agg = sum of 4-neighbor h ; h = relu(LN(agg @ Wl + bl) * g + be)
  xc = concat([x, h1, h2, h3, h4])  # [1024, 1056]
  z  = relu(BN(xc @ W1 + b1))       # BN eval-mode affine
  y  = z @ W2 + b2 ; out = where(mask, y, -1e7)

Implementation notes:
 - activations feature-major (FM) [feat, tok]; grid aggregation fused into
   the matmul PSUM accumulation: vertical +-32-token shifts via shifted
   stationary-operand slices over zero guard bands; horizontal +-1 neighbors
   pre-summed on GpSimd (hh).
 - act-stationary matmuls give node-major z blocks [128 tok, 256]; LN stats
   per block-pair (bn_stats/bn_aggr + sqrt + reciprocal), normalize via
   tensor_scalar, PE-transpose back to FM, ACT applies gamma/beta + relu.
 - samples processed in interleaved pairs so one sample's matmuls cover the
   other's LayerNorm chain (keeps the PE warm).
 - matmul operand dtype: float32r (full PE rate at moving dim >= 256) or
   bfloat16 (enables fast weight load), selected by USE_BF16.
"""

import numpy as np

import concourse.bass as bass
import concourse.bacc as bacc
import concourse.mybir as mybir
import concourse.tile as tile
from concourse.bass_utils import run_bass_kernel_spmd
from concourse.masks import make_identity

FP = mybir.dt.float32
FR = mybir.dt.float32r
BF = mybir.dt.bfloat16
AF = mybir.ActivationFunctionType
OP = mybir.AluOpType

GRID = 32
NN = 1024            # nodes per sample
F_IN = 32
H = 256
B = 64
S = 8                # samples per core
NCORE = 8
NB = 8               # 128-token blocks per sample
OBS_W = NN + NN * F_IN   # 33792
MIN_VAL = -10000000.0
EPS_LN = 1e-5
EPS_BN = 1e-5
PAD = 32             # token guard band for vertical shifts
HW = NN + 2 * PAD    # 1088, padded token width per feature-half

USE_BF16 = False
PROFILE = False
LAST_EXEC_NS = None
TRACE_KWARGS = {}


def _build(has_gin_bias: bool, b2_val: float, use_bf16: bool) -> bass.Bass:
    nc = bacc.Bacc("TRN2", target_bir_lowering=False, debug=False)

    MT = BF if use_bf16 else FP          # storage dtype of matmul operands
    GI = mybir.dt.uint16 if use_bf16 else mybir.dt.uint32

    def mm(ap):
        """View an operand/producer AP in the matmul dtype."""
        return ap if use_bf16 else ap.bitcast(FR)

    obs = nc.declare_dram_parameter("obs", [S, OBS_W], FP, isOutput=False)
    w0 = nc.declare_dram_parameter("w0", [F_IN, H], FP, isOutput=False)
    ws = nc.declare_dram_parameter("ws", [3, 2, 128, H], FP, isOutput=False)
    w1x = nc.declare_dram_parameter("w1x", [F_IN, 512], FP, isOutput=False)
    w1h = nc.declare_dram_parameter("w1h", [8, 128, 512], FP, isOutput=False)
    w2 = nc.declare_dram_parameter("w2", [4, 128], FP, isOutput=False)
    gg = nc.declare_dram_parameter("gg", [4, H], FP, isOutput=False)
    bb = nc.declare_dram_parameter("bb", [4, H], FP, isOutput=False)
    bns = nc.declare_dram_parameter("bns", [512], FP, isOutput=False)
    bnt = nc.declare_dram_parameter("bnt", [512], FP, isOutput=False)
    if has_gin_bias:
        gbias = nc.declare_dram_parameter("gbias", [4, H], FP, isOutput=False)
    y_out = nc.declare_dram_parameter("y", [S, NN], FP, isOutput=True)

    from contextlib import ExitStack

    with tile.TileContext(nc) as tc, ExitStack() as ctx:
        wp = ctx.enter_context(tc.tile_pool(name="w", bufs=1))
        px = ctx.enter_context(tc.tile_pool(name="px", bufs=2))
        ph = ctx.enter_context(tc.tile_pool(name="ph", bufs=2))
        pst = ctx.enter_context(tc.tile_pool(name="pst", bufs=8))
        pfin = ctx.enter_context(tc.tile_pool(name="pfin", bufs=1))
        pz = ctx.enter_context(tc.tile_pool(name="pz", bufs=3, space="PSUM"))
        ptf = ctx.enter_context(tc.tile_pool(name="ptf", bufs=5, space="PSUM"))

        # ---- constants / weights in SBUF ----
        ident = wp.tile([128, 128], MT, tag="id")
        make_identity(nc, ident[:])
        eps_sb = wp.tile([128, 1], FP, tag="eps")
        nc.gpsimd.memset(eps_sb[:], EPS_LN)

        w0_sb = wp.tile([F_IN, H], MT, tag="w0")
        nc.gpsimd.dma_start(mm(w0_sb[:]), w0[:, :])

        wl_sb = []
        for l in range(3):
            t = wp.tile([128, 2 * H], MT, tag=f"wl{l}")
            nc.gpsimd.dma_start(
                mm(t[:]).rearrange("p (k n) -> p k n", k=2),
                ws[l].rearrange("k p n -> p k n"),
            )
            wl_sb.append(t)

        w1x_sb = wp.tile([F_IN, 512], MT, tag="w1x")
        nc.gpsimd.dma_start(mm(w1x_sb[:]), w1x[:, :])
        w1h_sb = wp.tile([128, 8 * 512], MT, tag="w1h")
        nc.gpsimd.dma_start(
            mm(w1h_sb[:]).rearrange("p (j m) -> p j m", j=8),
            w1h[:, :, :].rearrange("j p m -> p j m"),
        )
        w2_sb = wp.tile([128, 4], MT, tag="w2")
        nc.gpsimd.dma_start(mm(w2_sb[:]), w2[:, :].rearrange("k p -> p k"))

        gg_sb = wp.tile([128, 8], FP, tag="gg")
        nc.sync.dma_start(
            gg_sb[:].rearrange("p (l c) -> p l c", c=2),
            gg[:, :].rearrange("l (c p) -> p l c", p=128),
        )
        bb_sb = wp.tile([128, 8], FP, tag="bb")
        nc.sync.dma_start(
            bb_sb[:].rearrange("p (l c) -> p l c", c=2),
            bb[:, :].rearrange("l (c p) -> p l c", p=128),
        )
        bns_sb = wp.tile([128, 4], FP, tag="bns")
        nc.sync.dma_start(bns_sb[:], bns[:].rearrange("(m p) -> p m", p=128))
        bnt_sb = wp.tile([128, 4], FP, tag="bnt")
        nc.sync.dma_start(bnt_sb[:], bnt[:].rearrange("(m p) -> p m", p=128))

        if has_gin_bias:
            ones1 = wp.tile([1, 128], MT, tag="ones1")
            if use_bf16:
                nc.gpsimd.memset(ones1[:].bitcast(mybir.dt.uint16), 0x3F80)
            else:
                nc.gpsimd.memset(ones1[:].bitcast(mybir.dt.uint32), 0x3F800000)
            gb_sb = wp.tile([1, 4 * H], MT, tag="gb")
            nc.gpsimd.dma_start(
                mm(gb_sb[:]).rearrange("q (l n) -> q l n", l=4), gbias[:, :]
            )

        def build_hh(hh_tile, src_tile, n_kc, pad):
            """hh[f, t] = h[f, left(t)] + h[f, right(t)]; on GpSimd."""
            for kc in range(n_kc):
                src = src_tile[:, kc * (NN + 2 * pad) + pad
                               : kc * (NN + 2 * pad) + pad + NN]
                dst = hh_tile[:, kc * NN : (kc + 1) * NN]
                sv = src.rearrange("p (r c) -> p r c", c=GRID)
                dv = dst.rearrange("p (r c) -> p r c", c=GRID)
                nc.gpsimd.tensor_add(
                    mm(dv[:, :, 1:31]), sv[:, :, 0:30], sv[:, :, 2:32]
                )
                nc.gpsimd.tensor_copy(mm(dv[:, :, 0:1]), sv[:, :, 1:2])
                nc.gpsimd.tensor_copy(mm(dv[:, :, 31:32]), sv[:, :, 30:31])

        def emit_layer_mms(z, b, lhs_tile, hh_tile, n_kc, rhs_of_kc, l):
            """Fused aggregation matmul group for one token block."""
            mms = []
            for kc in range(n_kc):
                mms.append(
                    (hh_tile[:, kc * NN + b * 128 : kc * NN + b * 128 + 128],
                     rhs_of_kc(kc))
                )
            for kc in range(n_kc):  # up neighbors (t-32)
                base = kc * HW + b * 128
                mms.append((lhs_tile[:, base : base + 128], rhs_of_kc(kc)))
            for kc in range(n_kc):  # down neighbors (t+32)
                base = kc * HW + b * 128 + 64
                mms.append((lhs_tile[:, base : base + 128], rhs_of_kc(kc)))
            n = len(mms) + (1 if has_gin_bias else 0)
            for i, (lhsT, rhs) in enumerate(mms):
                nc.tensor.matmul(
                    z[:, :], mm(lhsT), mm(rhs),
                    start=(i == 0), stop=(i == n - 1),
                )
            if has_gin_bias:
                nc.tensor.matmul(
                    z[:, :], mm(ones1[0:1, 0:128]),
                    mm(gb_sb[0:1, l * H : (l + 1) * H]),
                    start=False, stop=True,
                )

        def prep_x(s):
            """Load x for sample s, transpose to FM with guard bands."""
            x_nm = px.tile([128, 256], MT, tag="xnm")
            dma = nc.gpsimd.dma_start if use_bf16 else nc.sync.dma_start
            dma(
                x_nm[:].rearrange("p (b f) -> p b f", f=F_IN),
                obs[s, NN:OBS_W].rearrange("(b p f) -> p b f", p=128, f=F_IN),
            )
            x_fm = px.tile([F_IN, HW], MT, tag="xfm")
            nc.gpsimd.memset(x_fm[:, 0:PAD].bitcast(GI), 0)
            nc.gpsimd.memset(x_fm[:, PAD + NN : HW].bitcast(GI), 0)
            for half in range(2):
                x_tfm = ptf.tile([F_IN, 512], MT, tag="tf")
                for i in range(4):
                    b = half * 4 + i
                    nc.tensor.transpose(
                        x_tfm[:, i * 128 : (i + 1) * 128],
                        x_nm[:, b * F_IN : (b + 1) * F_IN],
                        ident[:],
                    )
                nc.scalar.copy(
                    mm(x_fm[:, PAD + half * 512 : PAD + (half + 1) * 512]),
                    x_tfm[:],
                )
            hh_x = px.tile([F_IN, NN], MT, tag="hhx")
            build_hh(hh_x, x_fm, 1, PAD)
            return {"s": s, "x_fm": x_fm, "hh_x": hh_x, "h": []}

        def layer_mm_phase(st, l):
            if l == 0:
                n_kc = 1
                prev, prev_hh = st["x_fm"], st["hh_x"]
                rhs_of_kc = lambda kc: w0_sb[:, :]
            else:
                n_kc = 2
                prev, prev_hh = st["h"][l - 1], st["hh"]
                wl = wl_sb[l - 1]
                rhs_of_kc = lambda kc, wl=wl: wl[:, kc * H : (kc + 1) * H]

            t_nm = ph.tile([128, NB * H], MT, tag="tnm")
            for bp in range(4):
                zs = []
                for b in (2 * bp, 2 * bp + 1):
                    z = pz.tile([128, H], FP, tag="z")
                    emit_layer_mms(z, b, prev, prev_hh, n_kc, rhs_of_kc, l)
                    zs.append(z)
                mvp = pst.tile([128, 4], FP, tag="mv")
                for i, z in enumerate(zs):
                    st6 = pst.tile([128, 6], FP, tag="st6")
                    nc.vector.bn_stats(st6[:], z[:, :])
                    nc.vector.bn_aggr(mvp[:, 2 * i : 2 * i + 2], st6[:])
                sdp = pst.tile([128, 2], FP, tag="sd")
                var_view = mvp[:].rearrange("p (b t) -> p t b", t=2)[:, 1, :]
                nc.scalar.activation(
                    sdp[:], var_view, AF.Sqrt, bias=eps_sb[:, 0:1], scale=1.0
                )
                invp = pst.tile([128, 2], FP, tag="inv")
                nc.vector.reciprocal(invp[:], sdp[:])
                for i, z in enumerate(zs):
                    b = 2 * bp + i
                    nc.vector.tensor_scalar(
                        out=t_nm[:, b * H : (b + 1) * H],
                        in0=z[:, :],
                        scalar1=mvp[:, 2 * i : 2 * i + 1],
                        scalar2=invp[:, i : i + 1],
                        op0=OP.subtract,
                        op1=OP.mult,
                    )
            st["t_nm"] = t_nm

        def layer_tr_phase(st, l):
            t_nm = st.pop("t_nm")
            h_t = ph.tile([128, 2 * HW], MT, tag=f"h{l}")
            nc.gpsimd.memset(h_t[:, 0:PAD].bitcast(GI), 0)
            nc.gpsimd.memset(h_t[:, PAD + NN : HW + PAD].bitcast(GI), 0)
            nc.gpsimd.memset(h_t[:, HW + PAD + NN : 2 * HW].bitcast(GI), 0)
            for half in range(2):
                for c in range(2):
                    tf = ptf.tile([128, 512], MT, tag="tf", name="tfc")
                    for i in range(4):
                        b = half * 4 + i
                        nc.tensor.transpose(
                            tf[:, i * 128 : (i + 1) * 128],
                            t_nm[:, b * H + c * 128 : b * H + c * 128 + 128],
                            ident[:],
                        )
                    nc.scalar.activation(
                        mm(h_t[:, c * HW + PAD + half * 512
                               : c * HW + PAD + (half + 1) * 512]),
                        tf[:],
                        AF.Relu,
                        scale=gg_sb[:, l * 2 + c : l * 2 + c + 1],
                        bias=bb_sb[:, l * 2 + c : l * 2 + c + 1],
                    )
            st["h"].append(h_t)
            if l < 3:
                hh_t = ph.tile([128, 2 * NN], MT, tag="hh")
                build_hh(hh_t, h_t, 2, PAD)
                st["hh"] = hh_t

        def unit_w1(st):
            z_sb = ph.tile([128, 4096], MT, tag="zsb")
            for m in range(4):
                for c2 in range(2):
                    zw1 = ptf.tile([128, 512], FP, tag="tf")
                    for kc in range(9):
                        if kc == 0:
                            lhsT = w1x_sb[:, m * 128 : (m + 1) * 128]
                            rt, roff = st["x_fm"], 0
                        else:
                            j = kc - 1
                            lhsT = w1h_sb[:, j * 512 + m * 128
                                          : j * 512 + (m + 1) * 128]
                            rt, roff = st["h"][j // 2], (j % 2) * HW
                        nc.tensor.matmul(
                            zw1[:, :],
                            mm(lhsT),
                            mm(rt[:, roff + PAD + c2 * 512
                                   : roff + PAD + (c2 + 1) * 512]),
                            start=(kc == 0), stop=(kc == 8),
                        )
                    nc.scalar.activation(
                        mm(z_sb[:, m * NN + c2 * 512 : m * NN + (c2 + 1) * 512]),
                        zw1[:],
                        AF.Relu,
                        scale=bns_sb[:, m : m + 1],
                        bias=bnt_sb[:, m : m + 1],
                    )
            st["z_sb"] = z_sb

        def unit_w2(st):
            s = st["s"]
            z_sb = st["z_sb"]
            y_s = pfin.tile([1, NN], FP, tag="ys", bufs=2)
            for c2 in range(2):
                yp = pz.tile([1, 512], FP, tag="z")
                for m in range(4):
                    nc.tensor.matmul(
                        yp[0:1, :],
                        mm(w2_sb[:, m : m + 1]),
                        mm(z_sb[:, m * NN + c2 * 512 : m * NN + (c2 + 1) * 512]),
                        start=(m == 0), stop=(m == 3),
                    )
                nc.vector.tensor_copy(y_s[:, c2 * 512 : (c2 + 1) * 512], yp[0:1, :])
            if b2_val != 0.0:
                nc.scalar.add(y_s[:], y_s[:], b2_val)
            m_s = pfin.tile([1, NN], FP, tag="ms", bufs=2)
            nc.sync.dma_start(m_s[:], obs[s : s + 1, 0:NN])
            yf = pfin.tile([1, NN], FP, tag="yfin", bufs=2)
            nc.gpsimd.memset(yf[:], MIN_VAL)
            nc.vector.copy_predicated(yf[:], m_s[:].bitcast(mybir.dt.uint32), y_s[:])
            nc.sync.dma_start(y_out[s : s + 1, :], yf[:])

        # ---- interleaved sample pairs: partner matmuls hide LN latency.
        # Both samples' matmul phases are emitted before either sample's
        # transpose phase so the PE instruction stream never waits on the
        # just-issued LayerNorm chain.
        for p in range(S // 2):
            sts = [prep_x(2 * p), prep_x(2 * p + 1)]
            for l in range(4):
                for st in sts:
                    layer_mm_phase(st, l)
                for st in sts:
                    layer_tr_phase(st, l)
            for st in sts:
                unit_w1(st)
            for st in sts:
                unit_w2(st)

    nc.finalize()
    return nc


_BUILD_CACHE = {}


def _get_nc(has_gin_bias: bool, b2_val: float, use_bf16: bool) -> bass.Bass:
    key = (has_gin_bias, float(b2_val), use_bf16)
    if key not in _BUILD_CACHE:
        _BUILD_CACHE[key] = _build(has_gin_bias, b2_val, use_bf16)
    return _BUILD_CACHE[key]


def prep_maps(observations, W0, b0, g0, be0, Ws, bs, gs, bes,
              W1, b1, bn_g, bn_b, bn_m, bn_v, W2, b2, **_ignored):
    obs = np.ascontiguousarray(np.asarray(observations, np.float32))
    W0 = np.ascontiguousarray(np.asarray(W0, np.float32))
    Ws = np.asarray(Ws, np.float32)
    W1 = np.asarray(W1, np.float32)
    W2 = np.asarray(W2, np.float32)
    gg = np.ascontiguousarray(np.stack(
        [np.asarray(g0, np.float32)] + [np.asarray(gs, np.float32)[i] for i in range(3)]))
    bb = np.ascontiguousarray(np.stack(
        [np.asarray(be0, np.float32)] + [np.asarray(bes, np.float32)[i] for i in range(3)]))
    gbias = np.ascontiguousarray(np.stack(
        [np.asarray(b0, np.float32)] + [np.asarray(bs, np.float32)[i] for i in range(3)]))
    has_gin_bias = bool(np.any(gbias != 0.0))
    bn_scale = (np.asarray(bn_g, np.float32)
                / np.sqrt(np.asarray(bn_v, np.float32) + EPS_BN)).astype(np.float32)
    bn_shift = ((np.asarray(b1, np.float32) - np.asarray(bn_m, np.float32)) * bn_scale
                + np.asarray(bn_b, np.float32)).astype(np.float32)
    b2_val = float(np.asarray(b2, np.float32).reshape(-1)[0])

    ws_r = np.ascontiguousarray(Ws.reshape(3, 2, 128, H))
    w1x = np.ascontiguousarray(W1[:F_IN])
    w1h = np.ascontiguousarray(W1[F_IN:].reshape(8, 128, 512))
    w2r = np.ascontiguousarray(W2.reshape(4, 128))

    shared = {
        "w0": W0, "ws": ws_r, "w1x": w1x, "w1h": w1h, "w2": w2r,
        "gg": gg, "bb": bb, "bns": bn_scale, "bnt": bn_shift,
    }
    if has_gin_bias:
        shared["gbias"] = gbias
    in_maps = []
    for c in range(NCORE):
        m = dict(shared)
        m["obs"] = np.ascontiguousarray(obs[c * S : (c + 1) * S])
        in_maps.append(m)
    return in_maps, has_gin_bias, b2_val


def kernel(**inputs) -> np.ndarray:
    global LAST_EXEC_NS
    in_maps, has_gin_bias, b2_val = prep_maps(**inputs)
    nc = _get_nc(has_gin_bias, b2_val, USE_BF16)
    res = run_bass_kernel_spmd(
        nc, in_maps, list(range(NCORE)), trace=PROFILE, **TRACE_KWARGS
    )
    LAST_EXEC_NS = res.exec_time_ns
    y = np.concatenate([res.results[c]["y"] for c in range(NCORE)], axis=0)
    return y.reshape(B, NN).astype(np.float32)



# revision 2
# speedup vs baseline: 1.0661x; 1.0661x over previous
"""Trainium2 Bass kernel for the CherryAllocation NAGNN (grid GIN + MLP head).

Self-contained: hardcodes shapes/sharding. Data-parallel over batch:
64 samples -> 8 NeuronCores x 8 samples. Weights replicated.

Math per sample (grid 32x32, N=1024 nodes):
  mask = obs[:1024] != 0 ; x = obs[1024:].reshape(1024, 32)
  h0 = x
  for l in 0..3:  agg = sum of 4-neighbor h ; h = relu(LN(agg @ Wl + bl) * g + be)
  xc = concat([x, h1, h2, h3, h4])  # [1024, 1056]
  z  = relu(BN(xc @ W1 + b1))       # BN eval-mode affine
  y  = z @ W2 + b2 ; out = where(mask, y, -1e7)

Implementation notes:
 - activations feature-major (FM) [feat, tok]; grid aggregation fused into
   the matmul PSUM accumulation: vertical +-32-token shifts via shifted
   stationary-operand slices over zero guard bands; horizontal +-1 neighbors
   pre-summed on GpSimd (hh).
 - act-stationary matmuls give node-major z blocks [128 tok, 256]; LN stats
   per block-pair (bn_stats/bn_aggr + sqrt + reciprocal), normalize via
   tensor_scalar, PE-transpose back to FM, ACT applies gamma/beta + relu.
 - samples processed in interleaved pairs so one sample's matmuls cover the
   other's LayerNorm chain (keeps the PE warm).
 - matmul operand dtype: float32r (full PE rate at moving dim >= 256) or
   bfloat16 (enables fast weight load), selected by USE_BF16.
"""

import numpy as np

import concourse.bass as bass
import concourse.bacc as bacc
import concourse.mybir as mybir
import concourse.tile as tile
from concourse.bass_utils import run_bass_kernel_spmd
from concourse.masks import make_identity

FP = mybir.dt.float32
FR = mybir.dt.float32r
BF = mybir.dt.bfloat16
AF = mybir.ActivationFunctionType
OP = mybir.AluOpType

GRID = 32
NN = 1024            # nodes per sample
F_IN = 32
H = 256
B = 64
S = 8                # samples per core
NCORE = 8
NB = 8               # 128-token blocks per sample
OBS_W = NN + NN * F_IN   # 33792
MIN_VAL = -10000000.0
EPS_LN = 1e-5
EPS_BN = 1e-5
PAD = 32             # token guard band for vertical shifts
HW = NN + 2 * PAD    # 1088, padded token width per feature-half

USE_BF16 = True
PROFILE = False
LAST_EXEC_NS = None
TRACE_KWARGS = {}


def _build(has_gin_bias: bool, b2_val: float, use_bf16: bool) -> bass.Bass:
    nc = bacc.Bacc("TRN2", target_bir_lowering=False, debug=False)

    MT = BF if use_bf16 else FP          # storage dtype of matmul operands
    GI = mybir.dt.uint16 if use_bf16 else mybir.dt.uint32

    def mm(ap):
        """View an operand/producer AP in the matmul dtype."""
        return ap if use_bf16 else ap.bitcast(FR)

    obs = nc.declare_dram_parameter("obs", [S, OBS_W], FP, isOutput=False)
    w0 = nc.declare_dram_parameter("w0", [F_IN, H], FP, isOutput=False)
    ws = nc.declare_dram_parameter("ws", [3, 2, 128, H], FP, isOutput=False)
    w1x = nc.declare_dram_parameter("w1x", [F_IN, 512], FP, isOutput=False)
    w1h = nc.declare_dram_parameter("w1h", [8, 128, 512], FP, isOutput=False)
    w2 = nc.declare_dram_parameter("w2", [4, 128], FP, isOutput=False)
    gg = nc.declare_dram_parameter("gg", [4, H], FP, isOutput=False)
    bb = nc.declare_dram_parameter("bb", [4, H], FP, isOutput=False)
    bns = nc.declare_dram_parameter("bns", [512], FP, isOutput=False)
    bnt = nc.declare_dram_parameter("bnt", [512], FP, isOutput=False)
    if has_gin_bias:
        gbias = nc.declare_dram_parameter("gbias", [4, H], FP, isOutput=False)
    y_out = nc.declare_dram_parameter("y", [S, NN], FP, isOutput=True)

    from contextlib import ExitStack

    with tile.TileContext(nc) as tc, ExitStack() as ctx:
        wp = ctx.enter_context(tc.tile_pool(name="w", bufs=1))
        px = ctx.enter_context(tc.tile_pool(name="px", bufs=2))
        ph = ctx.enter_context(tc.tile_pool(name="ph", bufs=2))
        pst = ctx.enter_context(tc.tile_pool(name="pst", bufs=8))
        pfin = ctx.enter_context(tc.tile_pool(name="pfin", bufs=1))
        pz = ctx.enter_context(tc.tile_pool(name="pz", bufs=3, space="PSUM"))
        ptf = ctx.enter_context(tc.tile_pool(name="ptf", bufs=5, space="PSUM"))

        # ---- constants / weights in SBUF ----
        ident = wp.tile([128, 128], MT, tag="id")
        make_identity(nc, ident[:])
        eps_sb = wp.tile([128, 1], FP, tag="eps")
        nc.gpsimd.memset(eps_sb[:], EPS_LN)

        w0_sb = wp.tile([F_IN, H], MT, tag="w0")
        nc.gpsimd.dma_start(mm(w0_sb[:]), w0[:, :])

        wl_sb = []
        for l in range(3):
            t = wp.tile([128, 2 * H], MT, tag=f"wl{l}")
            nc.gpsimd.dma_start(
                mm(t[:]).rearrange("p (k n) -> p k n", k=2),
                ws[l].rearrange("k p n -> p k n"),
            )
            wl_sb.append(t)

        w1x_sb = wp.tile([F_IN, 512], MT, tag="w1x")
        nc.gpsimd.dma_start(mm(w1x_sb[:]), w1x[:, :])
        w1h_sb = wp.tile([128, 8 * 512], MT, tag="w1h")
        nc.gpsimd.dma_start(
            mm(w1h_sb[:]).rearrange("p (j m) -> p j m", j=8),
            w1h[:, :, :].rearrange("j p m -> p j m"),
        )
        w2_sb = wp.tile([128, 4], MT, tag="w2")
        nc.gpsimd.dma_start(mm(w2_sb[:]), w2[:, :].rearrange("k p -> p k"))

        gg_sb = wp.tile([128, 8], FP, tag="gg")
        nc.sync.dma_start(
            gg_sb[:].rearrange("p (l c) -> p l c", c=2),
            gg[:, :].rearrange("l (c p) -> p l c", p=128),
        )
        bb_sb = wp.tile([128, 8], FP, tag="bb")
        nc.sync.dma_start(
            bb_sb[:].rearrange("p (l c) -> p l c", c=2),
            bb[:, :].rearrange("l (c p) -> p l c", p=128),
        )
        bns_sb = wp.tile([128, 4], FP, tag="bns")
        nc.sync.dma_start(bns_sb[:], bns[:].rearrange("(m p) -> p m", p=128))
        bnt_sb = wp.tile([128, 4], FP, tag="bnt")
        nc.sync.dma_start(bnt_sb[:], bnt[:].rearrange("(m p) -> p m", p=128))

        if has_gin_bias:
            ones1 = wp.tile([1, 128], MT, tag="ones1")
            if use_bf16:
                nc.gpsimd.memset(ones1[:].bitcast(mybir.dt.uint16), 0x3F80)
            else:
                nc.gpsimd.memset(ones1[:].bitcast(mybir.dt.uint32), 0x3F800000)
            gb_sb = wp.tile([1, 4 * H], MT, tag="gb")
            nc.gpsimd.dma_start(
                mm(gb_sb[:]).rearrange("q (l n) -> q l n", l=4), gbias[:, :]
            )

        def build_hh(hh_tile, src_tile, n_kc, pad):
            """hh[f, t] = h[f, left(t)] + h[f, right(t)]; on GpSimd."""
            for kc in range(n_kc):
                src = src_tile[:, kc * (NN + 2 * pad) + pad
                               : kc * (NN + 2 * pad) + pad + NN]
                dst = hh_tile[:, kc * NN : (kc + 1) * NN]
                sv = src.rearrange("p (r c) -> p r c", c=GRID)
                dv = dst.rearrange("p (r c) -> p r c", c=GRID)
                nc.gpsimd.tensor_add(
                    mm(dv[:, :, 1:31]), sv[:, :, 0:30], sv[:, :, 2:32]
                )
                nc.gpsimd.tensor_copy(mm(dv[:, :, 0:1]), sv[:, :, 1:2])
                nc.gpsimd.tensor_copy(mm(dv[:, :, 31:32]), sv[:, :, 30:31])

        def emit_layer_mms(z, b, lhs_tile, hh_tile, n_kc, rhs_of_kc, l):
            """Fused aggregation matmul group for one token block."""
            mms = []
            for kc in range(n_kc):
                mms.append(
                    (hh_tile[:, kc * NN + b * 128 : kc * NN + b * 128 + 128],
                     rhs_of_kc(kc))
                )
            for kc in range(n_kc):  # up neighbors (t-32)
                base = kc * HW + b * 128
                mms.append((lhs_tile[:, base : base + 128], rhs_of_kc(kc)))
            for kc in range(n_kc):  # down neighbors (t+32)
                base = kc * HW + b * 128 + 64
                mms.append((lhs_tile[:, base : base + 128], rhs_of_kc(kc)))
            n = len(mms) + (1 if has_gin_bias else 0)
            for i, (lhsT, rhs) in enumerate(mms):
                nc.tensor.matmul(
                    z[:, :], mm(lhsT), mm(rhs),
                    start=(i == 0), stop=(i == n - 1),
                )
            if has_gin_bias:
                nc.tensor.matmul(
                    z[:, :], mm(ones1[0:1, 0:128]),
                    mm(gb_sb[0:1, l * H : (l + 1) * H]),
                    start=False, stop=True,
                )

        def prep_x(s):
            """Load x for sample s, transpose to FM with guard bands."""
            x_nm = px.tile([128, 256], MT, tag="xnm")
            dma = nc.gpsimd.dma_start if use_bf16 else nc.sync.dma_start
            dma(
                x_nm[:].rearrange("p (b f) -> p b f", f=F_IN),
                obs[s, NN:OBS_W].rearrange("(b p f) -> p b f", p=128, f=F_IN),
            )
            x_fm = px.tile([F_IN, HW], MT, tag="xfm")
            nc.gpsimd.memset(x_fm[:, 0:PAD].bitcast(GI), 0)
            nc.gpsimd.memset(x_fm[:, PAD + NN : HW].bitcast(GI), 0)
            for half in range(2):
                x_tfm = ptf.tile([F_IN, 512], MT, tag="tf")
                for i in range(4):
                    b = half * 4 + i
                    nc.tensor.transpose(
                        x_tfm[:, i * 128 : (i + 1) * 128],
                        x_nm[:, b * F_IN : (b + 1) * F_IN],
                        ident[:],
                    )
                nc.scalar.copy(
                    mm(x_fm[:, PAD + half * 512 : PAD + (half + 1) * 512]),
                    x_tfm[:],
                )
            hh_x = px.tile([F_IN, NN], MT, tag="hhx")
            build_hh(hh_x, x_fm, 1, PAD)
            return {"s": s, "x_fm": x_fm, "hh_x": hh_x, "h": []}

        def layer_mm_phase(st, l):
            if l == 0:
                n_kc = 1
                prev, prev_hh = st["x_fm"], st["hh_x"]
                rhs_of_kc = lambda kc: w0_sb[:, :]
            else:
                n_kc = 2
                prev, prev_hh = st["h"][l - 1], st["hh"]
                wl = wl_sb[l - 1]
                rhs_of_kc = lambda kc, wl=wl: wl[:, kc * H : (kc + 1) * H]

            t_nm = ph.tile([128, NB * H], MT, tag="tnm")
            for bp in range(4):
                zs = []
                for b in (2 * bp, 2 * bp + 1):
                    z = pz.tile([128, H], FP, tag="z")
                    emit_layer_mms(z, b, prev, prev_hh, n_kc, rhs_of_kc, l)
                    zs.append(z)
                mvp = pst.tile([128, 4], FP, tag="mv")
                for i, z in enumerate(zs):
                    st6 = pst.tile([128, 6], FP, tag="st6")
                    nc.vector.bn_stats(st6[:], z[:, :])
                    nc.vector.bn_aggr(mvp[:, 2 * i : 2 * i + 2], st6[:])
                sdp = pst.tile([128, 2], FP, tag="sd")
                var_view = mvp[:].rearrange("p (b t) -> p t b", t=2)[:, 1, :]
                nc.scalar.activation(
                    sdp[:], var_view, AF.Sqrt, bias=eps_sb[:, 0:1], scale=1.0
                )
                invp = pst.tile([128, 2], FP, tag="inv")
                nc.vector.reciprocal(invp[:], sdp[:])
                for i, z in enumerate(zs):
                    b = 2 * bp + i
                    nc.vector.tensor_scalar(
                        out=t_nm[:, b * H : (b + 1) * H],
                        in0=z[:, :],
                        scalar1=mvp[:, 2 * i : 2 * i + 1],
                        scalar2=invp[:, i : i + 1],
                        op0=OP.subtract,
                        op1=OP.mult,
                    )
            st["t_nm"] = t_nm

        def layer_tr_phase(st, l):
            t_nm = st.pop("t_nm")
            h_t = ph.tile([128, 2 * HW], MT, tag=f"h{l}")
            nc.gpsimd.memset(h_t[:, 0:PAD].bitcast(GI), 0)
            nc.gpsimd.memset(h_t[:, PAD + NN : HW + PAD].bitcast(GI), 0)
            nc.gpsimd.memset(h_t[:, HW + PAD + NN : 2 * HW].bitcast(GI), 0)
            for half in range(2):
                for c in range(2):
                    tf = ptf.tile([128, 512], MT, tag="tf", name="tfc")
                    for i in range(4):
                        b = half * 4 + i
                        nc.tensor.transpose(
                            tf[:, i * 128 : (i + 1) * 128],
                            t_nm[:, b * H + c * 128 : b * H + c * 128 + 128],
                            ident[:],
                        )
                    nc.scalar.activation(
                        mm(h_t[:, c * HW + PAD + half * 512
                               : c * HW + PAD + (half + 1) * 512]),
                        tf[:],
                        AF.Relu,
                        scale=gg_sb[:, l * 2 + c : l * 2 + c + 1],
                        bias=bb_sb[:, l * 2 + c : l * 2 + c + 1],
                    )
            st["h"].append(h_t)
            if l < 3:
                hh_t = ph.tile([128, 2 * NN], MT, tag="hh")
                build_hh(hh_t, h_t, 2, PAD)
                st["hh"] = hh_t

        def unit_w1(st):
            z_sb = ph.tile([128, 4096], MT, tag="zsb")
            for m in range(4):
                for c2 in range(2):
                    zw1 = ptf.tile([128, 512], FP, tag="tf")
                    for kc in range(9):
                        if kc == 0:
                            lhsT = w1x_sb[:, m * 128 : (m + 1) * 128]
                            rt, roff = st["x_fm"], 0
                        else:
                            j = kc - 1
                            lhsT = w1h_sb[:, j * 512 + m * 128
                                          : j * 512 + (m + 1) * 128]
                            rt, roff = st["h"][j // 2], (j % 2) * HW
                        nc.tensor.matmul(
                            zw1[:, :],
                            mm(lhsT),
                            mm(rt[:, roff + PAD + c2 * 512
                                   : roff + PAD + (c2 + 1) * 512]),
                            start=(kc == 0), stop=(kc == 8),
                        )
                    nc.scalar.activation(
                        mm(z_sb[:, m * NN + c2 * 512 : m * NN + (c2 + 1) * 512]),
                        zw1[:],
                        AF.Relu,
                        scale=bns_sb[:, m : m + 1],
                        bias=bnt_sb[:, m : m + 1],
                    )
            st["z_sb"] = z_sb

        def unit_w2(st):
            s = st["s"]
            z_sb = st["z_sb"]
            y_s = pfin.tile([1, NN], FP, tag="ys", bufs=2)
            for c2 in range(2):
                yp = pz.tile([1, 512], FP, tag="z")
                for m in range(4):
                    nc.tensor.matmul(
                        yp[0:1, :],
                        mm(w2_sb[:, m : m + 1]),
                        mm(z_sb[:, m * NN + c2 * 512 : m * NN + (c2 + 1) * 512]),
                        start=(m == 0), stop=(m == 3),
                    )
                nc.vector.tensor_copy(y_s[:, c2 * 512 : (c2 + 1) * 512], yp[0:1, :])
            if b2_val != 0.0:
                nc.scalar.add(y_s[:], y_s[:], b2_val)
            m_s = pfin.tile([1, NN], FP, tag="ms", bufs=2)
            nc.sync.dma_start(m_s[:], obs[s : s + 1, 0:NN])
            yf = pfin.tile([1, NN], FP, tag="yfin", bufs=2)
            nc.gpsimd.memset(yf[:], MIN_VAL)
            nc.vector.copy_predicated(yf[:], m_s[:].bitcast(mybir.dt.uint32), y_s[:])
            nc.sync.dma_start(y_out[s : s + 1, :], yf[:])

        # ---- interleaved sample pairs: partner matmuls hide LN latency.
        # Both samples' matmul phases are emitted before either sample's
        # transpose phase so the PE instruction stream never waits on the
        # just-issued LayerNorm chain.
        for p in range(S // 2):
            sts = [prep_x(2 * p), prep_x(2 * p + 1)]
            for l in range(4):
                for st in sts:
                    layer_mm_phase(st, l)
                for st in sts:
                    layer_tr_phase(st, l)
            for st in sts:
                unit_w1(st)
            for st in sts:
                unit_w2(st)

    nc.finalize()
    return nc


_BUILD_CACHE = {}


def _get_nc(has_gin_bias: bool, b2_val: float, use_bf16: bool) -> bass.Bass:
    key = (has_gin_bias, float(b2_val), use_bf16)
    if key not in _BUILD_CACHE:
        _BUILD_CACHE[key] = _build(has_gin_bias, b2_val, use_bf16)
    return _BUILD_CACHE[key]


def prep_maps(observations, W0, b0, g0, be0, Ws, bs, gs, bes,
              W1, b1, bn_g, bn_b, bn_m, bn_v, W2, b2, **_ignored):
    obs = np.ascontiguousarray(np.asarray(observations, np.float32))
    W0 = np.ascontiguousarray(np.asarray(W0, np.float32))
    Ws = np.asarray(Ws, np.float32)
    W1 = np.asarray(W1, np.float32)
    W2 = np.asarray(W2, np.float32)
    gg = np.ascontiguousarray(np.stack(
        [np.asarray(g0, np.float32)] + [np.asarray(gs, np.float32)[i] for i in range(3)]))
    bb = np.ascontiguousarray(np.stack(
        [np.asarray(be0, np.float32)] + [np.asarray(bes, np.float32)[i] for i in range(3)]))
    gbias = np.ascontiguousarray(np.stack(
        [np.asarray(b0, np.float32)] + [np.asarray(bs, np.float32)[i] for i in range(3)]))
    has_gin_bias = bool(np.any(gbias != 0.0))
    bn_scale = (np.asarray(bn_g, np.float32)
                / np.sqrt(np.asarray(bn_v, np.float32) + EPS_BN)).astype(np.float32)
    bn_shift = ((np.asarray(b1, np.float32) - np.asarray(bn_m, np.float32)) * bn_scale
                + np.asarray(bn_b, np.float32)).astype(np.float32)
    b2_val = float(np.asarray(b2, np.float32).reshape(-1)[0])

    ws_r = np.ascontiguousarray(Ws.reshape(3, 2, 128, H))
    w1x = np.ascontiguousarray(W1[:F_IN])
    w1h = np.ascontiguousarray(W1[F_IN:].reshape(8, 128, 512))
    w2r = np.ascontiguousarray(W2.reshape(4, 128))

    shared = {
        "w0": W0, "ws": ws_r, "w1x": w1x, "w1h": w1h, "w2": w2r,
        "gg": gg, "bb": bb, "bns": bn_scale, "bnt": bn_shift,
    }
    if has_gin_bias:
        shared["gbias"] = gbias
    in_maps = []
    for c in range(NCORE):
        m = dict(shared)
        m["obs"] = np.ascontiguousarray(obs[c * S : (c + 1) * S])
        in_maps.append(m)
    return in_maps, has_gin_bias, b2_val


def kernel(**inputs) -> np.ndarray:
    global LAST_EXEC_NS
    in_maps, has_gin_bias, b2_val = prep_maps(**inputs)
    nc = _get_nc(has_gin_bias, b2_val, USE_BF16)
    res = run_bass_kernel_spmd(
        nc, in_maps, list(range(NCORE)), trace=PROFILE, **TRACE_KWARGS
    )
    LAST_EXEC_NS = res.exec_time_ns
    y = np.concatenate([res.results[c]["y"] for c in range(NCORE)], axis=0)
    return y.reshape(B, NN).astype(np.float32)



# revision 11
# speedup vs baseline: 1.3167x; 1.2351x over previous
"""Trainium2 Bass kernel for the CherryAllocation NAGNN (grid GIN + MLP head).

Self-contained: hardcodes shapes/sharding. Data-parallel over batch:
64 samples -> 8 NeuronCores x 8 samples. Weights replicated.

Math per sample (grid 32x32, N=1024 nodes):
  mask = obs[:1024] != 0 ; x = obs[1024:].reshape(1024, 32)
  h0 = x
  for l in 0..3:  agg = sum of 4-neighbor h ; h = relu(LN(agg @ Wl + bl) * g + be)
  xc = concat([x, h1, h2, h3, h4])  # [1024, 1056]
  z  = relu(BN(xc @ W1 + b1))       # BN eval-mode affine
  y  = z @ W2 + b2 ; out = where(mask, y, -1e7)

Implementation notes:
 - all matmul operands bf16, pre-converted on CPU (no converting DMAs).
 - LN mean is folded into the weights (W' = W - rowmean(W)), so z is
   centered by construction; LN reduces to z * rsqrt(mean(z^2) + eps),
   computed with one vector tensor_tensor_reduce per z block, sqrt on the
   act engine and vector reciprocal.  The normalize multiply runs on the
   act engine (per-partition scale) as the PSUM->SBUF copy.
 - activations feature-major (FM) [feat, tok]; grid aggregation fused into
   the matmul PSUM accumulation: vertical +-32-token shifts via shifted
   stationary-operand slices over zero guard bands; horizontal +-1 neighbors
   pre-summed on GpSimd (hh).  hh-dependent matmuls are emitted last so the
   GpSimd latency hides under the up/down matmuls.
 - samples processed in interleaved pairs so one sample's matmuls cover the
   other's LayerNorm chain; the next pair's x load/transpose is emitted
   inside the current pair's W1 phase.
 - guard-band memsets are only emitted the first time each round-robin
   buffer is used; afterwards the bands are already zero.
"""

import numpy as np

import concourse.bass as bass
import concourse.bacc as bacc
import concourse.mybir as mybir
import concourse.tile as tile
from concourse.bass_utils import run_bass_kernel_spmd
from concourse.masks import make_identity

FP = mybir.dt.float32
BF = mybir.dt.bfloat16
AF = mybir.ActivationFunctionType
OP = mybir.AluOpType

GRID = 32
NN = 1024            # nodes per sample
F_IN = 32
H = 256
B = 64
S = 8                # samples per core
NCORE = 8
NB = 8               # 128-token blocks per sample
OBS_W = NN + NN * F_IN   # 33792
MIN_VAL = -10000000.0
EPS_LN = 1e-5
EPS_BN = 1e-5
PAD = 32             # token guard band for vertical shifts
HW = NN + 2 * PAD    # 1088, padded token width per feature-half

PROFILE = False
LAST_EXEC_NS = None
TRACE_KWARGS = {}


def _build(has_gin_bias: bool, b2_val: float) -> bass.Bass:
    nc = bacc.Bacc("TRN2", target_bir_lowering=False, debug=False)

    xf = nc.declare_dram_parameter("xf", [S, 128, 256], BF, isOutput=False)
    msk = nc.declare_dram_parameter("msk", [S, NN], FP, isOutput=False)
    w0 = nc.declare_dram_parameter("w0", [F_IN, H], BF, isOutput=False)
    ws = nc.declare_dram_parameter("ws", [3, 2, 128, H], BF, isOutput=False)
    w1x = nc.declare_dram_parameter("w1x", [F_IN, 512], BF, isOutput=False)
    w1h = nc.declare_dram_parameter("w1h", [8, 128, 512], BF, isOutput=False)
    w2 = nc.declare_dram_parameter("w2", [4, 128], BF, isOutput=False)
    gg = nc.declare_dram_parameter("gg", [4, H], FP, isOutput=False)
    bb = nc.declare_dram_parameter("bb", [4, H], FP, isOutput=False)
    bns = nc.declare_dram_parameter("bns", [512], FP, isOutput=False)
    bnt = nc.declare_dram_parameter("bnt", [512], FP, isOutput=False)
    if has_gin_bias:
        gbias = nc.declare_dram_parameter("gbias", [4, H], BF, isOutput=False)
    y_out = nc.declare_dram_parameter("y", [S, NN], FP, isOutput=True)

    from contextlib import ExitStack

    with tile.TileContext(nc) as tc, ExitStack() as ctx:
        wp = ctx.enter_context(tc.tile_pool(name="w", bufs=1))
        px = ctx.enter_context(tc.tile_pool(name="px", bufs=4))
        ph = ctx.enter_context(tc.tile_pool(name="ph", bufs=2))
        pst = ctx.enter_context(tc.tile_pool(name="pst", bufs=8))
        pfin = ctx.enter_context(tc.tile_pool(name="pfin", bufs=1))
        pz = ctx.enter_context(tc.tile_pool(name="pz", bufs=4, space="PSUM"))
        ptf = ctx.enter_context(tc.tile_pool(name="ptf", bufs=4, space="PSUM"))

        # ---- constants / weights in SBUF ----
        ident = wp.tile([128, 128], BF, tag="id")
        make_identity(nc, ident[:])
        eps_sb = wp.tile([128, 1], FP, tag="eps")
        nc.gpsimd.memset(eps_sb[:], EPS_LN)

        w0_sb = wp.tile([F_IN, H], BF, tag="w0")
        nc.gpsimd.dma_start(w0_sb[:], w0[:, :])

        wl_sb = []
        for l in range(3):
            t = wp.tile([128, 2 * H], BF, tag=f"wl{l}")
            nc.gpsimd.dma_start(
                t[:].rearrange("p (k n) -> p k n", k=2),
                ws[l].rearrange("k p n -> p k n"),
            )
            wl_sb.append(t)

        w1x_sb = wp.tile([F_IN, 512], BF, tag="w1x")
        nc.gpsimd.dma_start(w1x_sb[:], w1x[:, :])
        w1h_sb = wp.tile([128, 8 * 512], BF, tag="w1h")
        nc.gpsimd.dma_start(
            w1h_sb[:].rearrange("p (j m) -> p j m", j=8),
            w1h[:, :, :].rearrange("j p m -> p j m"),
        )
        w2_sb = wp.tile([128, 4], BF, tag="w2")
        nc.gpsimd.dma_start(w2_sb[:], w2[:, :].rearrange("k p -> p k"))

        gg_sb = wp.tile([128, 8], FP, tag="gg")
        nc.sync.dma_start(
            gg_sb[:].rearrange("p (l c) -> p l c", c=2),
            gg[:, :].rearrange("l (c p) -> p l c", p=128),
        )
        bb_sb = wp.tile([128, 8], FP, tag="bb")
        nc.sync.dma_start(
            bb_sb[:].rearrange("p (l c) -> p l c", c=2),
            bb[:, :].rearrange("l (c p) -> p l c", p=128),
        )
        bns_sb = wp.tile([128, 4], FP, tag="bns")
        nc.sync.dma_start(bns_sb[:], bns[:].rearrange("(m p) -> p m", p=128))
        bnt_sb = wp.tile([128, 4], FP, tag="bnt")
        nc.sync.dma_start(bnt_sb[:], bnt[:].rearrange("(m p) -> p m", p=128))

        if has_gin_bias:
            ones1 = wp.tile([1, 128], BF, tag="ones1")
            nc.gpsimd.memset(ones1[:].bitcast(mybir.dt.uint16), 0x3F80)
            gb_sb = wp.tile([1, 4 * H], BF, tag="gb")
            nc.gpsimd.dma_start(
                gb_sb[:].rearrange("q (l n) -> q l n", l=4), gbias[:, :]
            )

        # persistent double-buffered activation tiles: guard bands zeroed
        # once here, only the [PAD:PAD+NN] spans are rewritten per use.
        h_pers = [[wp.tile([128, 2 * HW], BF, tag=f"h{l}p{j}",
                           name=f"hp{l}_{j}")
                   for j in range(2)] for l in range(4)]
        for l in range(4):
            for j in range(2):
                h_t = h_pers[l][j]
                nc.gpsimd.memset(h_t[:, 0:PAD].bitcast(mybir.dt.uint16), 0)
                nc.gpsimd.memset(
                    h_t[:, PAD + NN : HW + PAD].bitcast(mybir.dt.uint16), 0)
                nc.gpsimd.memset(
                    h_t[:, HW + PAD + NN : 2 * HW].bitcast(mybir.dt.uint16), 0)
        xfm_pers = [wp.tile([F_IN, HW], BF, tag=f"xfm{j}", name=f"xfmp{j}")
                    for j in range(4)]
        for j in range(4):
            nc.gpsimd.memset(xfm_pers[j][:, 0:PAD].bitcast(mybir.dt.uint16), 0)
            nc.gpsimd.memset(
                xfm_pers[j][:, PAD + NN : HW].bitcast(mybir.dt.uint16), 0)

        def build_hh(hh_tile, src_tile, n_kc, pad):
            """hh[f, t] = h[f, left(t)] + h[f, right(t)]; on GpSimd."""
            for kc in range(n_kc):
                src = src_tile[:, kc * (NN + 2 * pad) + pad
                               : kc * (NN + 2 * pad) + pad + NN]
                dst = hh_tile[:, kc * NN : (kc + 1) * NN]
                sv = src.rearrange("p (r c) -> p r c", c=GRID)
                dv = dst.rearrange("p (r c) -> p r c", c=GRID)
                nc.gpsimd.tensor_add(
                    dv[:, :, 1:31], sv[:, :, 0:30], sv[:, :, 2:32]
                )
                nc.gpsimd.tensor_copy(dv[:, :, 0:1], sv[:, :, 1:2])
                nc.gpsimd.tensor_copy(dv[:, :, 31:32], sv[:, :, 30:31])

        def emit_layer_mms(z, b, lhs_tile, hh_tile, n_kc, rhs_of_kc, l):
            """Fused aggregation matmul group for one token block.

            hh-dependent matmuls go last so the GpSimd hh build of the
            previous transpose phase is off the critical path."""
            mms = []
            for kc in range(n_kc):  # up neighbors (t-32)
                base = kc * HW + b * 128
                mms.append((lhs_tile[:, base : base + 128], rhs_of_kc(kc)))
            for kc in range(n_kc):  # down neighbors (t+32)
                base = kc * HW + b * 128 + 64
                mms.append((lhs_tile[:, base : base + 128], rhs_of_kc(kc)))
            for kc in range(n_kc):  # horizontal pair, pre-summed on GpSimd
                mms.append(
                    (hh_tile[:, kc * NN + b * 128 : kc * NN + b * 128 + 128],
                     rhs_of_kc(kc))
                )
            n = len(mms) + (1 if has_gin_bias else 0)
            for i, (lhsT, rhs) in enumerate(mms):
                nc.tensor.matmul(
                    z[:, :], lhsT, rhs,
                    start=(i == 0), stop=(i == n - 1),
                )
            if has_gin_bias:
                nc.tensor.matmul(
                    z[:, :], ones1[0:1, 0:128],
                    gb_sb[0:1, l * H : (l + 1) * H],
                    start=False, stop=True,
                )

        def prep_x(s):
            """Load x for sample s, transpose to FM with guard bands."""
            x_nm = px.tile([128, 256], BF, tag="xnm")
            nc.sync.dma_start(x_nm[:], xf[s])
            x_fm = xfm_pers[s % 4]
            for half in range(2):
                x_tfm = ptf.tile([F_IN, 512], BF, tag="tf")
                for i in range(4):
                    b = half * 4 + i
                    nc.tensor.transpose(
                        x_tfm[:, i * 128 : (i + 1) * 128],
                        x_nm[:, b * F_IN : (b + 1) * F_IN],
                        ident[:],
                    )
                nc.scalar.copy(
                    x_fm[:, PAD + half * 512 : PAD + (half + 1) * 512],
                    x_tfm[:],
                )
            hh_x = px.tile([F_IN, NN], BF, tag="hhx")
            build_hh(hh_x, x_fm, 1, PAD)
            return {"s": s, "x_fm": x_fm, "hh_x": hh_x, "h": []}

        def layer_mm_phase(st, l):
            if l == 0:
                n_kc = 1
                prev, prev_hh = st["x_fm"], st["hh_x"]
                rhs_of_kc = lambda kc: w0_sb[:, :]
            else:
                n_kc = 2
                prev, prev_hh = st["h"][l - 1], st["hh"]
                wl = wl_sb[l - 1]
                rhs_of_kc = lambda kc, wl=wl: wl[:, kc * H : (kc + 1) * H]

            t_nm = ph.tile([128, NB * H], BF, tag="tnm")
            for bp in range(4):
                zs = []
                for b in (2 * bp, 2 * bp + 1):
                    z = pz.tile([128, H], FP, tag="z")
                    emit_layer_mms(z, b, prev, prev_hh, n_kc, rhs_of_kc, l)
                    zs.append(z)
                mvp = pst.tile([128, 4], FP, tag="mv")
                for i, z in enumerate(zs):
                    st6 = pst.tile([128, 6], FP, tag="st6")
                    nc.vector.bn_stats(st6[:], z[:, :])
                    nc.vector.bn_aggr(mvp[:, 2 * i : 2 * i + 2], st6[:])
                sdp = pst.tile([128, 2], FP, tag="sd")
                var_view = mvp[:].rearrange("p (b t) -> p t b", t=2)[:, 1, :]
                nc.scalar.activation(sdp[:], var_view, AF.Sqrt,
                                     bias=eps_sb[:, 0:1], scale=1.0)
                invp = pst.tile([128, 2], FP, tag="inv")
                nc.vector.reciprocal(invp[:], sdp[:])
                for i, z in enumerate(zs):
                    b = 2 * bp + i
                    nc.scalar.activation(
                        t_nm[:, b * H : (b + 1) * H], z[:, :],
                        AF.Copy, bias=0.0, scale=invp[:, i : i + 1],
                    )
            st["t_nm"] = t_nm

        def layer_tr_phase(st, l):
            t_nm = st.pop("t_nm")
            h_t = h_pers[l][st["s"] % 2]
            for half in range(2):
                for c in range(2):
                    tf = ptf.tile([128, 512], BF, tag="tf", name="tfc")
                    for i in range(4):
                        b = half * 4 + i
                        nc.tensor.transpose(
                            tf[:, i * 128 : (i + 1) * 128],
                            t_nm[:, b * H + c * 128 : b * H + c * 128 + 128],
                            ident[:],
                        )
                    nc.scalar.activation(
                        h_t[:, c * HW + PAD + half * 512
                            : c * HW + PAD + (half + 1) * 512],
                        tf[:],
                        AF.Relu,
                        scale=gg_sb[:, l * 2 + c : l * 2 + c + 1],
                        bias=bb_sb[:, l * 2 + c : l * 2 + c + 1],
                    )
            st["h"].append(h_t)
            if l < 3:
                hh_t = ph.tile([128, 2 * NN], BF, tag="hh")
                build_hh(hh_t, h_t, 2, PAD)
                st["hh"] = hh_t

        def unit_w1(st):
            z_sb = ph.tile([128, 4096], BF, tag="zsb")
            for m in range(4):
                for c2 in range(2):
                    zw1 = ptf.tile([128, 512], FP, tag="tf")
                    for kc in range(9):
                        if kc == 0:
                            lhsT = w1x_sb[:, m * 128 : (m + 1) * 128]
                            rt, roff = st["x_fm"], 0
                        else:
                            j = kc - 1
                            lhsT = w1h_sb[:, j * 512 + m * 128
                                          : j * 512 + (m + 1) * 128]
                            rt, roff = st["h"][j // 2], (j % 2) * HW
                        nc.tensor.matmul(
                            zw1[:, :],
                            lhsT,
                            rt[:, roff + PAD + c2 * 512
                               : roff + PAD + (c2 + 1) * 512],
                            start=(kc == 0), stop=(kc == 8),
                        )
                    nc.scalar.activation(
                        z_sb[:, m * NN + c2 * 512 : m * NN + (c2 + 1) * 512],
                        zw1[:],
                        AF.Relu,
                        scale=bns_sb[:, m : m + 1],
                        bias=bnt_sb[:, m : m + 1],
                    )
            st["z_sb"] = z_sb

        def unit_w2(st):
            s = st["s"]
            z_sb = st["z_sb"]
            y_s = pfin.tile([1, NN], FP, tag="ys", bufs=2)
            for c2 in range(2):
                yp = pz.tile([1, 512], FP, tag="z")
                for m in range(4):
                    nc.tensor.matmul(
                        yp[0:1, :],
                        w2_sb[:, m : m + 1],
                        z_sb[:, m * NN + c2 * 512 : m * NN + (c2 + 1) * 512],
                        start=(m == 0), stop=(m == 3),
                    )
                nc.vector.tensor_copy(y_s[:, c2 * 512 : (c2 + 1) * 512], yp[0:1, :])
            if b2_val != 0.0:
                nc.scalar.add(y_s[:], y_s[:], b2_val)
            m_s = pfin.tile([1, NN], FP, tag="ms", bufs=2)
            nc.sync.dma_start(m_s[:], msk[s : s + 1, :])
            yf = pfin.tile([1, NN], FP, tag="yfin", bufs=2)
            nc.gpsimd.memset(yf[:], MIN_VAL)
            nc.vector.copy_predicated(yf[:], m_s[:].bitcast(mybir.dt.uint32), y_s[:])
            nc.sync.dma_start(y_out[s : s + 1, :], yf[:])

        # ---- interleaved sample pairs: partner matmuls hide LN latency.
        # Both samples' matmul phases are emitted before either sample's
        # transpose phase so the PE instruction stream never waits on the
        # just-issued LayerNorm chain.  The next pair's x prep is emitted
        # between the two W1 units so its DMA + transposes hide under W1
        # matmuls.
        sts = [prep_x(0), prep_x(1)]
        for p in range(S // 2):
            for l in range(4):
                for st in sts:
                    layer_mm_phase(st, l)
                for st in sts:
                    layer_tr_phase(st, l)
            unit_w1(sts[0])
            nxt = ([prep_x(2 * p + 2), prep_x(2 * p + 3)]
                   if p < S // 2 - 1 else None)
            unit_w1(sts[1])
            for st in sts:
                unit_w2(st)
            sts = nxt

    nc.finalize()
    return nc


_BUILD_CACHE = {}


def _get_nc(has_gin_bias: bool, b2_val: float) -> bass.Bass:
    key = (has_gin_bias, float(b2_val))
    if key not in _BUILD_CACHE:
        _BUILD_CACHE[key] = _build(has_gin_bias, b2_val)
    return _BUILD_CACHE[key]


def prep_maps(observations, W0, b0, g0, be0, Ws, bs, gs, bes,
              W1, b1, bn_g, bn_b, bn_m, bn_v, W2, b2, **_ignored):
    import ml_dtypes
    BF_NP = ml_dtypes.bfloat16

    obs = np.asarray(observations, np.float32)
    W0 = np.asarray(W0, np.float64)
    Ws = np.asarray(Ws, np.float64)
    W1 = np.asarray(W1, np.float32)
    W2 = np.asarray(W2, np.float32)
    gg = np.ascontiguousarray(np.stack(
        [np.asarray(g0, np.float32)] + [np.asarray(gs, np.float32)[i] for i in range(3)]))
    bb = np.ascontiguousarray(np.stack(
        [np.asarray(be0, np.float32)] + [np.asarray(bes, np.float32)[i] for i in range(3)]))
    gbias = np.stack(
        [np.asarray(b0, np.float64)] + [np.asarray(bs, np.float64)[i] for i in range(3)])
    has_gin_bias = bool(np.any(gbias != 0.0))
    bn_scale = (np.asarray(bn_g, np.float32)
                / np.sqrt(np.asarray(bn_v, np.float32) + EPS_BN)).astype(np.float32)
    bn_shift = ((np.asarray(b1, np.float32) - np.asarray(bn_m, np.float32)) * bn_scale
                + np.asarray(bn_b, np.float32)).astype(np.float32)
    b2_val = float(np.asarray(b2, np.float32).reshape(-1)[0])

    # Fold the LayerNorm mean subtraction into the GIN weights: with
    # W' = W - rowmean(W) (and centered bias), z = agg @ W' + b' has zero
    # feature-mean, so LN only needs the second moment.
    W0c = W0 - W0.mean(axis=1, keepdims=True)
    Wsc = Ws - Ws.mean(axis=2, keepdims=True)
    gbc = gbias - gbias.mean(axis=1, keepdims=True)

    ws_r = np.ascontiguousarray(Wsc.reshape(3, 2, 128, H).astype(BF_NP))
    w0_r = np.ascontiguousarray(W0c.astype(BF_NP))
    w1x = np.ascontiguousarray(W1[:F_IN].astype(BF_NP))
    w1h = np.ascontiguousarray(W1[F_IN:].reshape(8, 128, 512).astype(BF_NP))
    w2r = np.ascontiguousarray(W2.reshape(4, 128).astype(BF_NP))

    shared = {
        "w0": w0_r, "ws": ws_r, "w1x": w1x, "w1h": w1h, "w2": w2r,
        "gg": gg, "bb": bb, "bns": bn_scale, "bnt": bn_shift,
    }
    if has_gin_bias:
        shared["gbias"] = np.ascontiguousarray(gbc.astype(BF_NP))
    in_maps = []
    for c in range(NCORE):
        m = dict(shared)
        ob = obs[c * S : (c + 1) * S]
        m["msk"] = np.ascontiguousarray(ob[:, :NN])
        # [S, 1024 tok, 32 f] -> token-block-major [S, 128 p, 8 blk, 32 f]
        m["xf"] = np.ascontiguousarray(
            ob[:, NN:].reshape(S, NB, 128, F_IN).transpose(0, 2, 1, 3)
            .reshape(S, 128, 256).astype(BF_NP))
        in_maps.append(m)
    return in_maps, has_gin_bias, b2_val


def kernel(**inputs) -> np.ndarray:
    global LAST_EXEC_NS
    in_maps, has_gin_bias, b2_val = prep_maps(**inputs)
    nc = _get_nc(has_gin_bias, b2_val)
    res = run_bass_kernel_spmd(
        nc, in_maps, list(range(NCORE)), trace=PROFILE, **TRACE_KWARGS
    )
    LAST_EXEC_NS = res.exec_time_ns
    y = np.concatenate([res.results[c]["y"] for c in range(NCORE)], axis=0)
    return y.reshape(B, NN).astype(np.float32)


# revision 26
# speedup vs baseline: 1.3384x; 1.0165x over previous
"""Trainium2 Bass kernel for the CherryAllocation NAGNN (grid GIN + MLP head).

Self-contained: hardcodes shapes/sharding. Data-parallel over batch:
64 samples -> 8 NeuronCores x 8 samples. Weights replicated.

Math per sample (grid 32x32, N=1024 nodes):
  mask = obs[:1024] != 0 ; x = obs[1024:].reshape(1024, 32)
  h0 = x
  for l in 0..3:  agg = sum of 4-neighbor h ; h = relu(LN(agg @ Wl + bl) * g + be)
  xc = concat([x, h1, h2, h3, h4])  # [1024, 1056]
  z  = relu(BN(xc @ W1 + b1))       # BN eval-mode affine
  y  = z @ W2 + b2 ; out = where(mask, y, -1e7)

Implementation notes:
 - all matmul operands bf16, pre-converted on CPU (no converting DMAs).
 - LN mean is folded into the weights (W' = W - rowmean(W)), so z is
   centered by construction; LN reduces to z * rsqrt(mean(z^2) + eps),
   computed with one vector tensor_tensor_reduce per z block, sqrt on the
   act engine and vector reciprocal.  The normalize multiply runs on the
   act engine (per-partition scale) as the PSUM->SBUF copy.
 - activations feature-major (FM) [feat, tok]; grid aggregation fused into
   the matmul PSUM accumulation: vertical +-32-token shifts via shifted
   stationary-operand slices over zero guard bands; horizontal +-1 neighbors
   pre-summed on GpSimd (hh).  hh-dependent matmuls are emitted last so the
   GpSimd latency hides under the up/down matmuls.
 - samples processed in interleaved pairs so one sample's matmuls cover the
   other's LayerNorm chain; the next pair's x load/transpose is emitted
   inside the current pair's W1 phase.
 - guard-band memsets are only emitted the first time each round-robin
   buffer is used; afterwards the bands are already zero.
"""

import numpy as np

import concourse.bass as bass
import concourse.bacc as bacc
import concourse.mybir as mybir
import concourse.tile as tile
from concourse.bass_utils import run_bass_kernel_spmd
from concourse.masks import make_identity

FP = mybir.dt.float32
BF = mybir.dt.bfloat16
AF = mybir.ActivationFunctionType
OP = mybir.AluOpType

GRID = 32
NN = 1024            # nodes per sample
F_IN = 32
H = 256
B = 64
S = 8                # samples per core
NCORE = 8
NB = 8               # 128-token blocks per sample
OBS_W = NN + NN * F_IN   # 33792
MIN_VAL = -10000000.0
EPS_LN = 1e-5
EPS_BN = 1e-5
PAD = 32             # token guard band for vertical shifts
HW = NN + 2 * PAD    # 1088, padded token width per feature-half

PROFILE = False
LAST_EXEC_NS = None
TRACE_KWARGS = {}


def _build(has_gin_bias: bool, b2_val: float) -> bass.Bass:
    nc = bacc.Bacc("TRN2", target_bir_lowering=False, debug=False)

    xf = nc.declare_dram_parameter("xf", [S, 128, 256], BF, isOutput=False)
    msk = nc.declare_dram_parameter("msk", [S, NN], FP, isOutput=False)
    w0 = nc.declare_dram_parameter("w0", [F_IN, H], BF, isOutput=False)
    ws = nc.declare_dram_parameter("ws", [3, 2, 128, H], BF, isOutput=False)
    w1x = nc.declare_dram_parameter("w1x", [F_IN, 512], BF, isOutput=False)
    w1h = nc.declare_dram_parameter("w1h", [8, 128, 512], BF, isOutput=False)
    w2 = nc.declare_dram_parameter("w2", [4, 128], BF, isOutput=False)
    gg = nc.declare_dram_parameter("gg", [4, H], FP, isOutput=False)
    bb = nc.declare_dram_parameter("bb", [4, H], FP, isOutput=False)
    bns = nc.declare_dram_parameter("bns", [512], FP, isOutput=False)
    bnt = nc.declare_dram_parameter("bnt", [512], FP, isOutput=False)
    if has_gin_bias:
        gbias = nc.declare_dram_parameter("gbias", [4, H], BF, isOutput=False)
    y_out = nc.declare_dram_parameter("y", [S, NN], FP, isOutput=True)

    from contextlib import ExitStack

    with tile.TileContext(nc) as tc, ExitStack() as ctx:
        wp = ctx.enter_context(tc.tile_pool(name="w", bufs=1))
        px = ctx.enter_context(tc.tile_pool(name="px", bufs=4))
        ph = ctx.enter_context(tc.tile_pool(name="ph", bufs=2))
        pst = ctx.enter_context(tc.tile_pool(name="pst", bufs=8))
        pfin = ctx.enter_context(tc.tile_pool(name="pfin", bufs=1))
        pz = ctx.enter_context(tc.tile_pool(name="pz", bufs=4, space="PSUM"))
        ptf = ctx.enter_context(tc.tile_pool(name="ptf", bufs=4, space="PSUM"))

        # ---- constants / weights in SBUF ----
        ident = wp.tile([128, 128], BF, tag="id")
        make_identity(nc, ident[:])
        eps_sb = wp.tile([128, 1], FP, tag="eps")
        nc.gpsimd.memset(eps_sb[:], EPS_LN)

        # weight DMAs trigger from the scalar queue (idle at startup) so
        # they don't serialize with gpsimd memsets / hh builds.
        w0_sb = wp.tile([F_IN, H], BF, tag="w0")
        nc.scalar.dma_start(w0_sb[:], w0[:, :])

        wl_sb = []
        for l in range(3):
            t = wp.tile([128, 2 * H], BF, tag=f"wl{l}")
            nc.scalar.dma_start(
                t[:].rearrange("p (k n) -> p k n", k=2),
                ws[l].rearrange("k p n -> p k n"),
            )
            wl_sb.append(t)

        w1x_sb = wp.tile([F_IN, 512], BF, tag="w1x")
        nc.scalar.dma_start(w1x_sb[:], w1x[:, :])
        w1h_sb = wp.tile([128, 8 * 512], BF, tag="w1h")
        nc.scalar.dma_start(
            w1h_sb[:].rearrange("p (j m) -> p j m", j=8),
            w1h[:, :, :].rearrange("j p m -> p j m"),
        )
        w2_sb = wp.tile([128, 4], BF, tag="w2")
        nc.scalar.dma_start(w2_sb[:], w2[:, :].rearrange("k p -> p k"))

        gg_sb = wp.tile([128, 8], FP, tag="gg")
        nc.sync.dma_start(
            gg_sb[:].rearrange("p (l c) -> p l c", c=2),
            gg[:, :].rearrange("l (c p) -> p l c", p=128),
        )
        bb_sb = wp.tile([128, 8], FP, tag="bb")
        nc.sync.dma_start(
            bb_sb[:].rearrange("p (l c) -> p l c", c=2),
            bb[:, :].rearrange("l (c p) -> p l c", p=128),
        )
        bns_sb = wp.tile([128, 4], FP, tag="bns")
        nc.sync.dma_start(bns_sb[:], bns[:].rearrange("(m p) -> p m", p=128))
        bnt_sb = wp.tile([128, 4], FP, tag="bnt")
        nc.sync.dma_start(bnt_sb[:], bnt[:].rearrange("(m p) -> p m", p=128))

        if has_gin_bias:
            ones1 = wp.tile([1, 128], BF, tag="ones1")
            nc.gpsimd.memset(ones1[:].bitcast(mybir.dt.uint16), 0x3F80)
            gb_sb = wp.tile([1, 4 * H], BF, tag="gb")
            nc.gpsimd.dma_start(
                gb_sb[:].rearrange("q (l n) -> q l n", l=4), gbias[:, :]
            )

        def build_hh(hh_tile, src_tile, n_kc):
            """hh[f, t] = h[f, left(t)] + h[f, right(t)]; on GpSimd."""
            for kc in range(n_kc):
                src = src_tile[:, kc * HW + PAD : kc * HW + PAD + NN]
                dst = hh_tile[:, kc * NN : (kc + 1) * NN]
                sv = src.rearrange("p (r c) -> p r c", c=GRID)
                dv = dst.rearrange("p (r c) -> p r c", c=GRID)
                nc.gpsimd.tensor_add(
                    dv[:, :, 1:31], sv[:, :, 0:30], sv[:, :, 2:32]
                )
                nc.gpsimd.tensor_copy(dv[:, :, 0:1], sv[:, :, 1:2])
                nc.gpsimd.tensor_copy(dv[:, :, 31:32], sv[:, :, 30:31])

        def emit_layer_mms(z, b, lhs_tile, hh_tile, n_kc, rhs_of_kc, l):
            """Fused aggregation matmul group for one token block.

            hh-dependent matmuls go last so the GpSimd hh build of the
            previous transpose phase is off the critical path."""
            mms = []
            for kc in range(n_kc):  # up neighbors (t-32)
                base = kc * HW + b * 128
                mms.append((lhs_tile[:, base : base + 128], rhs_of_kc(kc)))
            for kc in range(n_kc):  # down neighbors (t+32)
                base = kc * HW + b * 128 + 64
                mms.append((lhs_tile[:, base : base + 128], rhs_of_kc(kc)))
            for kc in range(n_kc):  # horizontal pair, pre-summed on GpSimd
                mms.append(
                    (hh_tile[:, kc * NN + b * 128 : kc * NN + b * 128 + 128],
                     rhs_of_kc(kc))
                )
            n = len(mms) + (1 if has_gin_bias else 0)
            for i, (lhsT, rhs) in enumerate(mms):
                nc.tensor.matmul(
                    z[:, :], lhsT, rhs,
                    start=(i == 0), stop=(i == n - 1),
                )
            if has_gin_bias:
                nc.tensor.matmul(
                    z[:, :], ones1[0:1, 0:128],
                    gb_sb[0:1, l * H : (l + 1) * H],
                    start=False, stop=True,
                )

        def prep_x(s):
            """Load x for sample s, transpose to FM with guard bands."""
            x_nm = px.tile([128, 256], BF, tag="xnm")
            nc.sync.dma_start(x_nm[:], xf[s])
            x_fm = px.tile([F_IN, HW], BF, tag="xfm")
            nc.gpsimd.memset(x_fm[:, 0:PAD].bitcast(mybir.dt.uint16), 0)
            nc.gpsimd.memset(
                x_fm[:, PAD + NN : HW].bitcast(mybir.dt.uint16), 0)
            x_tfm = ptf.tile([F_IN, NN], BF, tag="tf")
            for b in range(8):
                nc.tensor.transpose(
                    x_tfm[:, b * 128 : (b + 1) * 128],
                    x_nm[:, b * F_IN : (b + 1) * F_IN],
                    ident[:],
                )
            nc.scalar.copy(x_fm[:, PAD : PAD + NN], x_tfm[:])
            hh_x = px.tile([F_IN, NN], BF, tag="hhx")
            build_hh(hh_x, x_fm, 1)
            return {"s": s, "x_fm": x_fm, "hh_x": hh_x, "h": []}

        def layer_mm_phase(st, l):
            if l == 0:
                n_kc = 1
                prev, prev_hh = st["x_fm"], st["hh_x"]
                rhs_of_kc = lambda kc: w0_sb[:, :]
            else:
                n_kc = 2
                prev, prev_hh = st["h"][l - 1], st["hh"]
                wl = wl_sb[l - 1]
                rhs_of_kc = lambda kc, wl=wl: wl[:, kc * H : (kc + 1) * H]

            t_nm = ph.tile([128, NB * H], BF, tag="tnm")
            for bp in range(4):
                zs = []
                for b in (2 * bp, 2 * bp + 1):
                    z = pz.tile([128, H], FP, tag="z")
                    emit_layer_mms(z, b, prev, prev_hh, n_kc, rhs_of_kc, l)
                    zs.append(z)
                mvp = pst.tile([128, 4], FP, tag="mv")
                for i, z in enumerate(zs):
                    st6 = pst.tile([128, 6], FP, tag="st6")
                    nc.vector.bn_stats(st6[:], z[:, :])
                    nc.vector.bn_aggr(mvp[:, 2 * i : 2 * i + 2], st6[:])
                sdp = pst.tile([128, 2], FP, tag="sd")
                var_view = mvp[:].rearrange("p (b t) -> p t b", t=2)[:, 1, :]
                nc.scalar.activation(sdp[:], var_view, AF.Sqrt,
                                     bias=eps_sb[:, 0:1], scale=1.0)
                invp = pst.tile([128, 2], FP, tag="inv")
                nc.vector.reciprocal(invp[:], sdp[:])
                for i, z in enumerate(zs):
                    b = 2 * bp + i
                    nc.scalar.activation(
                        t_nm[:, b * H : (b + 1) * H], z[:, :],
                        AF.Copy, bias=0.0, scale=invp[:, i : i + 1],
                    )
            st["t_nm"] = t_nm

        def layer_tr_phase(st, l):
            t_nm = st.pop("t_nm")
            h_t = ph.tile([128, 2 * HW], BF, tag=f"h{l}")
            nc.gpsimd.memset(h_t[:, 0:PAD].bitcast(mybir.dt.uint16), 0)
            nc.gpsimd.memset(
                h_t[:, PAD + NN : HW + PAD].bitcast(mybir.dt.uint16), 0)
            nc.gpsimd.memset(
                h_t[:, HW + PAD + NN : 2 * HW].bitcast(mybir.dt.uint16), 0)
            for c in range(2):
                tf = ptf.tile([128, NN], BF, tag="tf", name="tfc")
                for b in range(8):
                    nc.tensor.transpose(
                        tf[:, b * 128 : (b + 1) * 128],
                        t_nm[:, b * H + c * 128 : b * H + c * 128 + 128],
                        ident[:],
                    )
                nc.scalar.activation(
                    h_t[:, c * HW + PAD : c * HW + PAD + NN],
                    tf[:],
                    AF.Relu,
                    scale=gg_sb[:, l * 2 + c : l * 2 + c + 1],
                    bias=bb_sb[:, l * 2 + c : l * 2 + c + 1],
                )
            st["h"].append(h_t)
            if l < 3:
                hh_t = ph.tile([128, 2 * NN], BF, tag="hh")
                build_hh(hh_t, h_t, 2)
                st["hh"] = hh_t

        def unit_w1(st):
            z_sb = ph.tile([128, 4096], BF, tag="zsb")
            for m in range(4):
                for c2 in range(2):
                    zw1 = ptf.tile([128, 512], FP, tag="tf")
                    for kc in range(9):
                        if kc == 0:
                            lhsT = w1x_sb[:, m * 128 : (m + 1) * 128]
                            rt, roff = st["x_fm"], PAD
                        else:
                            j = kc - 1
                            lhsT = w1h_sb[:, j * 512 + m * 128
                                          : j * 512 + (m + 1) * 128]
                            rt, roff = st["h"][j // 2], (j % 2) * HW + PAD
                        nc.tensor.matmul(
                            zw1[:, :],
                            lhsT,
                            rt[:, roff + c2 * 512 : roff + (c2 + 1) * 512],
                            start=(kc == 0), stop=(kc == 8),
                        )
                    nc.scalar.activation(
                        z_sb[:, m * NN + c2 * 512 : m * NN + (c2 + 1) * 512],
                        zw1[:],
                        AF.Relu,
                        scale=bns_sb[:, m : m + 1],
                        bias=bnt_sb[:, m : m + 1],
                    )
            st["z_sb"] = z_sb

        def unit_w2(st):
            s = st["s"]
            z_sb = st["z_sb"]
            y_s = pfin.tile([1, NN], FP, tag="ys", bufs=2)
            for c2 in range(2):
                yp = pz.tile([1, 512], FP, tag="z")
                for m in range(4):
                    nc.tensor.matmul(
                        yp[0:1, :],
                        w2_sb[:, m : m + 1],
                        z_sb[:, m * NN + c2 * 512 : m * NN + (c2 + 1) * 512],
                        start=(m == 0), stop=(m == 3),
                    )
                nc.vector.tensor_copy(y_s[:, c2 * 512 : (c2 + 1) * 512], yp[0:1, :])
            if b2_val != 0.0:
                nc.scalar.add(y_s[:], y_s[:], b2_val)
            m_s = pfin.tile([1, NN], FP, tag="ms", bufs=2)
            nc.sync.dma_start(m_s[:], msk[s : s + 1, :])
            yf = pfin.tile([1, NN], FP, tag="yfin", bufs=2)
            nc.gpsimd.memset(yf[:], MIN_VAL)
            nc.vector.copy_predicated(yf[:], m_s[:].bitcast(mybir.dt.uint32), y_s[:])
            nc.sync.dma_start(y_out[s : s + 1, :], yf[:])

        # ---- interleaved sample pairs: partner matmuls hide LN latency.
        # Both samples' matmul phases are emitted before either sample's
        # transpose phase so the PE instruction stream never waits on the
        # just-issued LayerNorm chain.  The next pair's x prep is emitted
        # between the two W1 units so its DMA + transposes hide under W1
        # matmuls.
        sts = [prep_x(0), prep_x(1)]
        for p in range(S // 2):
            for l in range(4):
                for st in sts:
                    layer_mm_phase(st, l)
                for st in sts:
                    layer_tr_phase(st, l)
            unit_w1(sts[0])
            nxt = ([prep_x(2 * p + 2), prep_x(2 * p + 3)]
                   if p < S // 2 - 1 else None)
            unit_w1(sts[1])
            for st in sts:
                unit_w2(st)
            sts = nxt

    nc.finalize()
    return nc


_BUILD_CACHE = {}


def _get_nc(has_gin_bias: bool, b2_val: float) -> bass.Bass:
    key = (has_gin_bias, float(b2_val))
    if key not in _BUILD_CACHE:
        _BUILD_CACHE[key] = _build(has_gin_bias, b2_val)
    return _BUILD_CACHE[key]


def prep_maps(observations, W0, b0, g0, be0, Ws, bs, gs, bes,
              W1, b1, bn_g, bn_b, bn_m, bn_v, W2, b2, **_ignored):
    import ml_dtypes
    BF_NP = ml_dtypes.bfloat16

    obs = np.asarray(observations, np.float32)
    W0 = np.asarray(W0, np.float64)
    Ws = np.asarray(Ws, np.float64)
    W1 = np.asarray(W1, np.float32)
    W2 = np.asarray(W2, np.float32)
    gg = np.ascontiguousarray(np.stack(
        [np.asarray(g0, np.float32)] + [np.asarray(gs, np.float32)[i] for i in range(3)]))
    bb = np.ascontiguousarray(np.stack(
        [np.asarray(be0, np.float32)] + [np.asarray(bes, np.float32)[i] for i in range(3)]))
    gbias = np.stack(
        [np.asarray(b0, np.float64)] + [np.asarray(bs, np.float64)[i] for i in range(3)])
    has_gin_bias = bool(np.any(gbias != 0.0))
    bn_scale = (np.asarray(bn_g, np.float32)
                / np.sqrt(np.asarray(bn_v, np.float32) + EPS_BN)).astype(np.float32)
    bn_shift = ((np.asarray(b1, np.float32) - np.asarray(bn_m, np.float32)) * bn_scale
                + np.asarray(bn_b, np.float32)).astype(np.float32)
    b2_val = float(np.asarray(b2, np.float32).reshape(-1)[0])

    # Fold the LayerNorm mean subtraction into the GIN weights: with
    # W' = W - rowmean(W) (and centered bias), z = agg @ W' + b' has zero
    # feature-mean, so LN only needs the second moment.
    W0c = W0 - W0.mean(axis=1, keepdims=True)
    Wsc = Ws - Ws.mean(axis=2, keepdims=True)
    gbc = gbias - gbias.mean(axis=1, keepdims=True)

    ws_r = np.ascontiguousarray(Wsc.reshape(3, 2, 128, H).astype(BF_NP))
    w0_r = np.ascontiguousarray(W0c.astype(BF_NP))
    w1x = np.ascontiguousarray(W1[:F_IN].astype(BF_NP))
    w1h = np.ascontiguousarray(W1[F_IN:].reshape(8, 128, 512).astype(BF_NP))
    w2r = np.ascontiguousarray(W2.reshape(4, 128).astype(BF_NP))

    shared = {
        "w0": w0_r, "ws": ws_r, "w1x": w1x, "w1h": w1h, "w2": w2r,
        "gg": gg, "bb": bb, "bns": bn_scale, "bnt": bn_shift,
    }
    if has_gin_bias:
        shared["gbias"] = np.ascontiguousarray(gbc.astype(BF_NP))
    in_maps = []
    for c in range(NCORE):
        m = dict(shared)
        ob = obs[c * S : (c + 1) * S]
        m["msk"] = np.ascontiguousarray(ob[:, :NN])
        # [S, 1024 tok, 32 f] -> token-block-major [S, 128 p, 8 blk, 32 f]
        m["xf"] = np.ascontiguousarray(
            ob[:, NN:].reshape(S, NB, 128, F_IN).transpose(0, 2, 1, 3)
            .reshape(S, 128, 256).astype(BF_NP))
        in_maps.append(m)
    return in_maps, has_gin_bias, b2_val


def kernel(**inputs) -> np.ndarray:
    global LAST_EXEC_NS
    in_maps, has_gin_bias, b2_val = prep_maps(**inputs)
    nc = _get_nc(has_gin_bias, b2_val)
    res = run_bass_kernel_spmd(
        nc, in_maps, list(range(NCORE)), trace=PROFILE, **TRACE_KWARGS
    )
    LAST_EXEC_NS = res.exec_time_ns
    y = np.concatenate([res.results[c]["y"] for c in range(NCORE)], axis=0)
    return y.reshape(B, NN).astype(np.float32)
